# revision 12
# baseline (speedup 1.0000x reference)
"""Trainium2 Bass kernel for nn_CNN2Trans (sparse window attention + adapter).

Reference math (per batch b of 8; one batch per NeuronCore):
  q = xW_q' (scale folded);  kv[loc, n] = 2x2 window of x_cnn
  k|v = kv @ [Wk|Wv];  logits[loc,g,n] = sum_d q*k (+ q.bk fold)
  softmax_one over n=4;  ao = sum_n attn*v;  proj; residual; adapter MLP.

Layouts: activations loc-major (128 locations on partitions); x_trans is
host-transposed to feature-major [256, B] so projections can contract over
channels with data-stationary lhsT; output produced feature-major [256, B]
and host-transposed back.
"""

import sys
from contextlib import ExitStack

import numpy as np

sys.path.insert(0, "/opt/trn_rl_repo")

import concourse.bass as bass
import concourse.mybir as mybir
import concourse.tile as tile
from concourse import bacc
from concourse.bass_utils import run_bass_kernel_spmd
from concourse.masks import make_identity

F32 = mybir.dt.float32
F32R = mybir.dt.float32r
AX = mybir.AxisListType
OP = mybir.AluOpType
ACT = mybir.ActivationFunctionType

N_CORES = 8
L = 4096          # locations per core (one batch: 64*64)
NT = L // 128     # 32 attention tiles of 128 locs (2 rows of 64)
G, D, NW = 8, 32, 4
CQ, CKV, DH = 256, 128, 64

_cache = {}


def _ap(base, extra_free):
    """Rebuild AP with custom free dims: base partition dim + [(step, count)...]."""
    return bass.AP(
        tensor=base.tensor,
        offset=base.offset,
        ap=[base.ap[0]] + [[s, c] for (s, c) in extra_free],
    )


def _build(flags):
    has_qbias, has_cq, has_bv = flags
    nc = bacc.Bacc(
        "TRN2", target_bir_lowering=False, debug=False, num_devices=N_CORES
    )

    # -------- DRAM I/O (per-core shapes) --------
    xt = nc.dram_tensor("xt", [CQ, L], F32R, kind="ExternalInput")
    xc = nc.dram_tensor("xc", [CKV, 4 * L], F32R, kind="ExternalInput")
    wq = nc.dram_tensor("wq", [CQ, 264], F32R, kind="ExternalInput")
    wkv = nc.dram_tensor("wkv", [CKV, 512], F32R, kind="ExternalInput")
    wp = nc.dram_tensor("wp", [CQ, CQ], F32R, kind="ExternalInput")
    wa1 = nc.dram_tensor("wa1", [CQ, DH], F32R, kind="ExternalInput")
    wa2 = nc.dram_tensor("wa2", [DH, CQ], F32R, kind="ExternalInput")
    bqe = nc.dram_tensor("bqe", [264, 1], F32, kind="ExternalInput")
    bvv = nc.dram_tensor("bvv", [CQ, 1], F32, kind="ExternalInput")
    bpv = nc.dram_tensor("bpv", [CQ, 1], F32, kind="ExternalInput")
    ba1v = nc.dram_tensor("ba1v", [DH, 1], F32, kind="ExternalInput")
    ba2v = nc.dram_tensor("ba2v", [CQ, 1], F32, kind="ExternalInput")
    out = nc.dram_tensor("o", [CQ, L], F32, kind="ExternalOutput")

    with tile.TileContext(nc) as tc:
        with (
            tc.tile_pool(name="w", bufs=1) as wpool,
            tc.tile_pool(name="y", bufs=1) as ypool,
        ):
            # -------- stage weights --------
            wq_sb = wpool.tile([128, 2, 264], F32R)
            for c in range(2):
                nc.sync.dma_start(out=wq_sb[:, c, :], in_=wq[c * 128:(c + 1) * 128, :])
            wkv_sb = wpool.tile([128, 512], F32R)
            nc.sync.dma_start(out=wkv_sb, in_=wkv[:, :])
            wp_sb = wpool.tile([128, 2, 2, 128], F32R)
            for ci in range(2):
                for co in range(2):
                    nc.sync.dma_start(
                        out=wp_sb[:, ci, co, :],
                        in_=wp[ci * 128:(ci + 1) * 128, co * 128:(co + 1) * 128],
                    )
            wa1_sb = wpool.tile([128, 2, DH], F32R)
            for c in range(2):
                nc.sync.dma_start(out=wa1_sb[:, c, :], in_=wa1[c * 128:(c + 1) * 128, :])
            wa2_sb = wpool.tile([DH, 2, 128], F32R)
            for c in range(2):
                nc.sync.dma_start(out=wa2_sb[:, c, :], in_=wa2[:, c * 128:(c + 1) * 128])
            bp_sb = wpool.tile([128, 2], F32)
            for c in range(2):
                nc.sync.dma_start(out=bp_sb[:, c:c + 1], in_=bpv[c * 128:(c + 1) * 128, :])
            ba1_sb = wpool.tile([DH, 1], F32)
            nc.sync.dma_start(out=ba1_sb, in_=ba1v[:, :])
            ba2_sb = wpool.tile([128, 2], F32)
            for c in range(2):
                nc.sync.dma_start(out=ba2_sb[:, c:c + 1], in_=ba2v[c * 128:(c + 1) * 128, :])
            if has_qbias:
                bq_sb = wpool.tile([128, 264], F32)
                nc.sync.dma_start(
                    out=bq_sb,
                    in_=bass.AP(tensor=bqe, offset=0, ap=[[0, 128], [1, 264]]),
                )
            if has_bv:
                bv_sb = wpool.tile([128, 256], F32)
                nc.sync.dma_start(
                    out=bv_sb,
                    in_=bass.AP(tensor=bvv, offset=0, ap=[[0, 128], [1, 256]]),
                )
            ident = wpool.tile([128, 128], F32)
            make_identity(nc, ident)

            y_sb0 = ypool.tile([128, L], F32R, tag="y0")
            y_sb1 = ypool.tile([128, L], F32R, tag="y1")
            y_sb = [y_sb0, y_sb1]

            # ================= phase 1: attention + proj + residual ==========
            p1 = ExitStack()
            dpool = p1.enter_context(tc.tile_pool(name="d", bufs=3))
            apool = p1.enter_context(tc.tile_pool(name="a", bufs=2))
            spool = p1.enter_context(tc.tile_pool(name="sup", bufs=2))
            ppq = p1.enter_context(tc.tile_pool(name="pq", bufs=1, space="PSUM"))
            ppk = p1.enter_context(tc.tile_pool(name="pk", bufs=1, space="PSUM"))
            ppv = p1.enter_context(tc.tile_pool(name="pv", bufs=1, space="PSUM"))
            ppt = p1.enter_context(tc.tile_pool(name="pt", bufs=1, space="PSUM"))
            ppj = p1.enter_context(tc.tile_pool(name="pj", bufs=1, space="PSUM"))
            xt_tiles = [None, None]
            aoT_sb = None
            proj_ps = None
            for t in range(NT):
                st = t % 2  # subtile index within supertile
                xt_t = dpool.tile([128, 2, 128], F32R, tag="xt")
                xt_tiles[st] = xt_t
                for c in range(2):
                    nc.sync.dma_start(
                        out=xt_t[:, c, :],
                        in_=xt[c * 128:(c + 1) * 128, t * 128:(t + 1) * 128],
                    )
                # host pre-arranged: free = (n=(m,nn), loc=(r,w))
                kv_t = dpool.tile([128, NW, 128], F32R, tag="kv")
                nc.sync.dma_start(out=kv_t, in_=xc[:, t * 512:(t + 1) * 512])

                # ---- projections on PE (f32r) ----
                q_ps = ppq.tile([128, 264], F32, tag="qps")
                for c in range(2):
                    nc.tensor.matmul(
                        q_ps[:],
                        lhsT=xt_t[:, c, :],
                        rhs=wq_sb[:, c, :],
                        start=(c == 0),
                        stop=(c == 1),
                    )
                k_ps = ppk.tile([128, NW, 256], F32, tag="kps")
                v_ps = ppv.tile([128, NW, 256], F32, tag="vps")
                for n in range(NW):
                    nc.tensor.matmul(
                        k_ps[:, n, :], lhsT=kv_t[:, n, :], rhs=wkv_sb[:, 0:256]
                    )
                    nc.tensor.matmul(
                        v_ps[:, n, :], lhsT=kv_t[:, n, :], rhs=wkv_sb[:, 256:512]
                    )

                # ---- q evacuation (+ bias) ----
                q_sb = apool.tile([128, 264], F32, tag="qsb")
                if has_qbias:
                    nc.vector.scalar_tensor_tensor(
                        out=q_sb[:], in0=q_ps[:], scalar=1.0, in1=bq_sb[:],
                        op0=OP.bypass, op1=OP.add,
                    )
                else:
                    nc.vector.tensor_copy(q_sb[:], q_ps[:])

                # ---- QK: prods = k * q (broadcast over n), reduce over d ----
                prods = apool.tile([128, NW, 256], F32, tag="prods")
                nc.vector.tensor_mul(
                    prods[:], k_ps[:], _ap(q_sb[:], [(0, NW), (1, 256)])
                )
                logits = apool.tile([128, NW, G], F32, tag="logits")
                nc.vector.tensor_reduce(
                    out=logits[:],
                    in_=prods[:].rearrange("p n (g d) -> p n g d", g=G),
                    axis=AX.X,
                    op=OP.add,
                )
                if has_cq:
                    nc.vector.scalar_tensor_tensor(
                        out=logits[:], in0=logits[:], scalar=1.0,
                        in1=_ap(q_sb[:, 256:264], [(0, NW), (1, G)]),
                        op0=OP.bypass, op1=OP.add,
                    )

                # ---- softmax_one over n ----
                mx = apool.tile([128, G], F32, tag="mx")
                nc.vector.tensor_reduce(
                    out=mx[:], in_=logits[:].rearrange("p n g -> p g n"),
                    axis=AX.X, op=OP.max,
                )
                em = apool.tile([128, NW, G], F32, tag="em")
                nc.vector.scalar_tensor_tensor(
                    out=em[:], in0=logits[:], scalar=1.0,
                    in1=_ap(mx[:], [(0, NW), (1, G)]),
                    op0=OP.bypass, op1=OP.subtract,
                )
                e = apool.tile([128, NW, G], F32, tag="e")
                nc.scalar.activation(out=e[:], in_=em[:], func=ACT.Exp)
                se = apool.tile([128, G], F32, tag="se")
                nc.vector.tensor_reduce(
                    out=se[:], in_=e[:].rearrange("p n g -> p g n"),
                    axis=AX.X, op=OP.add,
                )
                sp = apool.tile([128, G], F32, tag="sp")
                nc.vector.tensor_scalar_add(sp[:], se[:], 1.0)
                rp = apool.tile([128, G], F32, tag="rp")
                nc.vector.reciprocal(rp[:], sp[:])
                attn = apool.tile([128, NW, G], F32, tag="attn")
                nc.vector.tensor_mul(
                    attn[:], e[:], _ap(rp[:], [(0, NW), (1, G)])
                )

                # ---- AV: prods2 = v * attn (broadcast over d), reduce over n ----
                prods2 = apool.tile([128, NW, 256], F32, tag="prods2")
                nc.vector.tensor_mul(
                    prods2[:].rearrange("p n (g d) -> p n g d", g=G),
                    v_ps[:].rearrange("p n (g d) -> p n g d", g=G),
                    _ap(attn[:], [(G, NW), (1, G), (0, D)]),
                )
                ao = apool.tile([128, 256], F32, tag="ao")
                nc.vector.tensor_reduce(
                    out=ao[:],
                    in_=prods2[:].rearrange("p n gd -> p gd n"),
                    axis=AX.X, op=OP.add,
                )
                if has_bv:
                    ssum = apool.tile([128, G], F32, tag="ssum")
                    nc.vector.tensor_mul(ssum[:], se[:], rp[:])
                    bvterm = apool.tile([128, 256], F32, tag="bvterm")
                    nc.vector.tensor_mul(
                        bvterm[:], bv_sb[:], _ap(ssum[:], [(1, G), (0, D)])
                    )
                    nc.vector.tensor_add(ao[:], ao[:], bvterm[:])

                # ---- transpose attention output to feature-major ----
                if st == 0:
                    aoT_sb = spool.tile([128, 2, 256], F32R, tag="aoT")
                aoT_ps = ppt.tile([128, 2, 128], F32, tag="aoTps")
                for c in range(2):
                    nc.tensor.transpose(
                        aoT_ps[:, c, :], ao[:, c * 128:(c + 1) * 128], ident[:]
                    )
                nc.scalar.activation(
                    out=aoT_sb[:, :, st * 128:(st + 1) * 128],
                    in_=aoT_ps[:],
                    func=ACT.Copy,
                )

                # ---- proj + residual per supertile (256 locs) ----
                if st == 1:
                    proj_ps = ppj.tile([128, 2, 256], F32, tag="pjps")
                    for co in range(2):
                        for ci in range(2):
                            nc.tensor.matmul(
                                proj_ps[:, co, :],
                                lhsT=wp_sb[:, ci, co, :],
                                rhs=aoT_sb[:, ci, :],
                                start=(ci == 0),
                                stop=(ci == 1),
                            )
                    for s2 in range(2):
                        tt = t - 1 + s2
                        for co in range(2):
                            nc.vector.scalar_tensor_tensor(
                                out=y_sb[co][:, tt * 128:(tt + 1) * 128],
                                in0=proj_ps[:, co, s2 * 128:(s2 + 1) * 128],
                                scalar=bp_sb[:, co:co + 1],
                                in1=xt_tiles[s2][:, co, :],
                                op0=OP.add, op1=OP.add,
                            )

            # ================= phase 2: adapter MLP ==========================
            p1.close()
            with (
                tc.tile_pool(name="p2", bufs=2) as p2pool,
                tc.tile_pool(name="ph", bufs=2, space="PSUM") as pph,
                tc.tile_pool(name="pa2", bufs=2, space="PSUM") as ppa2,
            ):
                for s in range(L // 256):
                    h_ps = pph.tile([DH, 256], F32, tag="hps")
                    for c in range(2):
                        nc.tensor.matmul(
                            h_ps[:],
                            lhsT=wa1_sb[:, c, :],
                            rhs=y_sb[c][:, s * 256:(s + 1) * 256],
                            start=(c == 0),
                            stop=(c == 1),
                        )
                    h_sb = p2pool.tile([DH, 256], F32R, tag="hsb")
                    nc.scalar.activation(
                        out=h_sb[:], in_=h_ps[:], func=ACT.Gelu, bias=ba1_sb[:, 0:1]
                    )
                    a2_ps = ppa2.tile([128, 2, 256], F32, tag="a2ps")
                    for co in range(2):
                        nc.tensor.matmul(
                            a2_ps[:, co, :],
                            lhsT=wa2_sb[:, co, :],
                            rhs=h_sb[:],
                        )
                    o_sb = p2pool.tile([128, 2, 256], F32, tag="osb")
                    for co in range(2):
                        nc.scalar.activation(
                            out=o_sb[:, co, :], in_=a2_ps[:, co, :],
                            func=ACT.Identity, bias=ba2_sb[:, co:co + 1],
                        )
                    for co in range(2):
                        nc.sync.dma_start(
                            out=out[co * 128:(co + 1) * 128, s * 256:(s + 1) * 256],
                            in_=o_sb[:, co, :],
                        )
    nc.finalize()
    return nc


def kernel(x_trans, x_cnn, Wq, bq, Wk, bk, Wv, bv, Wp, bp, Wa1, ba1, Wa2, ba2,
           trace=False):
    scale = float(D) ** -0.5
    wq_s = (Wq * scale).astype(np.float32)
    bq_s = (bq * scale).astype(np.float32)
    # fold of k-bias into logits: cq0 = q . bk per head, computed as 8 extra
    # q-projection columns (x @ Wcq + ccq)
    wcq = np.stack(
        [wq_s[:, g * D:(g + 1) * D] @ bk[g * D:(g + 1) * D] for g in range(G)], axis=1
    ).astype(np.float32)
    ccq = np.array(
        [bq_s[g * D:(g + 1) * D] @ bk[g * D:(g + 1) * D] for g in range(G)],
        dtype=np.float32,
    )
    wq_ext = np.ascontiguousarray(np.concatenate([wq_s, wcq], axis=1))
    bq_ext = np.concatenate([bq_s, ccq])
    wkv = np.ascontiguousarray(np.concatenate([Wk + 0.0, Wv + 0.0], axis=1)).astype(
        np.float32
    )

    has_qbias = bool(np.any(bq_ext != 0.0))
    has_cq = bool(np.any(wcq != 0.0) or np.any(ccq != 0.0))
    has_bv = bool(np.any(bv != 0.0))
    flags = (has_qbias, has_cq, has_bv)
    if flags not in _cache:
        _cache[flags] = _build(flags)
    nc = _cache[flags]

    common = {
        "wq": wq_ext,
        "wkv": wkv,
        "wp": np.ascontiguousarray(Wp, dtype=np.float32),
        "wa1": np.ascontiguousarray(Wa1, dtype=np.float32),
        "wa2": np.ascontiguousarray(Wa2, dtype=np.float32),
        "bqe": np.ascontiguousarray(bq_ext.reshape(264, 1), dtype=np.float32),
        "bvv": np.ascontiguousarray(bv.reshape(CQ, 1), dtype=np.float32),
        "bpv": np.ascontiguousarray(bp.reshape(CQ, 1), dtype=np.float32),
        "ba1v": np.ascontiguousarray(ba1.reshape(DH, 1), dtype=np.float32),
        "ba2v": np.ascontiguousarray(ba2.reshape(CQ, 1), dtype=np.float32),
    }
    in_maps = []
    for b in range(N_CORES):
        m = dict(common)
        m["xt"] = np.ascontiguousarray(
            x_trans[b].reshape(L, CQ).T, dtype=np.float32
        )
        # [c, hh, ww] -> [c, t, m, nn, r, w]; tile t covers h rows (2t, 2t+1)
        xcb = x_cnn[b].reshape(CKV, 32, 2, 2, 64, 2)  # c, t, r, m, w, nn
        xcb = xcb.transpose(0, 1, 3, 5, 2, 4).reshape(CKV, 4 * L)
        m["xc"] = np.ascontiguousarray(xcb, dtype=np.float32)
        in_maps.append(m)

    res = run_bass_kernel_spmd(
        nc, in_maps, core_ids=list(range(N_CORES)), trace=trace
    )
    kernel.last_results = res
    o = np.stack([r["o"] for r in res.results])  # [8, 256, 4096]
    return (
        o.transpose(0, 2, 1).reshape(8, 64, 64, CQ).astype(np.float32)
    )


# revision 16
# speedup vs baseline: 1.5201x; 1.5201x over previous
"""Trainium2 Bass kernel for nn_CNN2Trans (sparse window attention + adapter).

Reference math (per batch b of 8; one batch per NeuronCore):
  q = xW_q' (scale folded);  kv[loc, n] = 2x2 window of x_cnn
  k|v = kv @ [Wk|Wv];  logits[loc,g,n] = sum_d q*k (+ q.bk fold)
  softmax_one over n=4;  ao = sum_n attn*v;  proj; residual; adapter MLP.

Layouts: activations loc-major (128 locations on partitions); x_trans is
host-transposed to feature-major [256, B] so projections can contract over
channels with data-stationary lhsT; output produced feature-major [256, B]
and host-transposed back.
"""

import sys
from contextlib import ExitStack

import numpy as np

sys.path.insert(0, "/opt/trn_rl_repo")

import concourse.bass as bass
import concourse.mybir as mybir
import concourse.tile as tile
from concourse import bacc
from concourse.bass_utils import run_bass_kernel_spmd
from concourse.masks import make_identity

F32 = mybir.dt.float32
F32R = mybir.dt.float32r
AX = mybir.AxisListType
OP = mybir.AluOpType
ACT = mybir.ActivationFunctionType

N_CORES = 8
L = 4096          # locations per core (one batch: 64*64)
NT = L // 128     # 32 attention tiles of 128 locs (2 rows of 64)
G, D, NW = 8, 32, 4
CQ, CKV, DH = 256, 128, 64

_cache = {}


def _ap(base, extra_free):
    """Rebuild AP with custom free dims: base partition dim + [(step, count)...]."""
    return bass.AP(
        tensor=base.tensor,
        offset=base.offset,
        ap=[base.ap[0]] + [[s, c] for (s, c) in extra_free],
    )


def _build(flags):
    has_qbias, has_cq, has_bv = flags
    nc = bacc.Bacc(
        "TRN2", target_bir_lowering=False, debug=False, num_devices=N_CORES
    )

    # -------- DRAM I/O (per-core shapes) --------
    xt = nc.dram_tensor("xt", [CQ, L], F32R, kind="ExternalInput")
    xc = nc.dram_tensor("xc", [CKV, 4 * L], F32R, kind="ExternalInput")
    wq = nc.dram_tensor("wq", [CQ, 264], F32R, kind="ExternalInput")
    wkv = nc.dram_tensor("wkv", [CKV, 512], F32R, kind="ExternalInput")
    wp = nc.dram_tensor("wp", [CQ, CQ], F32R, kind="ExternalInput")
    wa1 = nc.dram_tensor("wa1", [CQ, DH], F32R, kind="ExternalInput")
    wa2 = nc.dram_tensor("wa2", [DH, CQ], F32R, kind="ExternalInput")
    bqe = nc.dram_tensor("bqe", [264, 1], F32, kind="ExternalInput")
    bvv = nc.dram_tensor("bvv", [CQ, 1], F32, kind="ExternalInput")
    bpv = nc.dram_tensor("bpv", [CQ, 1], F32, kind="ExternalInput")
    ba1v = nc.dram_tensor("ba1v", [DH, 1], F32, kind="ExternalInput")
    ba2v = nc.dram_tensor("ba2v", [CQ, 1], F32, kind="ExternalInput")
    eye = nc.dram_tensor("eye", [128, 128], F32R, kind="ExternalInput")
    out = nc.dram_tensor("o", [CQ, L], F32, kind="ExternalOutput")

    with tile.TileContext(nc) as tc:
        with (
            tc.tile_pool(name="w", bufs=1) as wpool,
            tc.tile_pool(name="y", bufs=1) as ypool,
        ):
            # -------- stage weights --------
            wq_sb = wpool.tile([128, 2, 264], F32R)
            for c in range(2):
                nc.sync.dma_start(out=wq_sb[:, c, :], in_=wq[c * 128:(c + 1) * 128, :])
            wkv_sb = wpool.tile([128, 512], F32R)
            nc.sync.dma_start(out=wkv_sb, in_=wkv[:, :])
            wp_sb = wpool.tile([128, 2, 2, 128], F32R)
            for ci in range(2):
                for co in range(2):
                    nc.sync.dma_start(
                        out=wp_sb[:, ci, co, :],
                        in_=wp[ci * 128:(ci + 1) * 128, co * 128:(co + 1) * 128],
                    )
            wa1_sb = wpool.tile([128, 2, DH], F32R)
            for c in range(2):
                nc.sync.dma_start(out=wa1_sb[:, c, :], in_=wa1[c * 128:(c + 1) * 128, :])
            wa2_sb = wpool.tile([DH, 2, 128], F32R)
            for c in range(2):
                nc.sync.dma_start(out=wa2_sb[:, c, :], in_=wa2[:, c * 128:(c + 1) * 128])
            bp_sb = wpool.tile([128, 2], F32)
            for c in range(2):
                nc.sync.dma_start(out=bp_sb[:, c:c + 1], in_=bpv[c * 128:(c + 1) * 128, :])
            ba1_sb = wpool.tile([DH, 1], F32)
            nc.sync.dma_start(out=ba1_sb, in_=ba1v[:, :])
            ba2_sb = wpool.tile([128, 2], F32)
            for c in range(2):
                nc.sync.dma_start(out=ba2_sb[:, c:c + 1], in_=ba2v[c * 128:(c + 1) * 128, :])
            if has_qbias:
                bq_sb = wpool.tile([128, 264], F32)
                nc.sync.dma_start(
                    out=bq_sb,
                    in_=bass.AP(tensor=bqe, offset=0, ap=[[0, 128], [1, 264]]),
                )
            if has_bv:
                bv_sb = wpool.tile([128, 256], F32)
                nc.sync.dma_start(
                    out=bv_sb,
                    in_=bass.AP(tensor=bvv, offset=0, ap=[[0, 128], [1, 256]]),
                )
            ident = wpool.tile([128, 128], F32)
            make_identity(nc, ident)
            eye_sb = wpool.tile([128, 128], F32R)
            nc.sync.dma_start(out=eye_sb, in_=eye[:, :])

            y_sb0 = ypool.tile([128, L], F32R, tag="y0")
            y_sb1 = ypool.tile([128, L], F32R, tag="y1")
            y_sb = [y_sb0, y_sb1]

            # ================= phase 1: attention + proj + residual ==========
            p1 = ExitStack()
            dpool = p1.enter_context(tc.tile_pool(name="d", bufs=3))
            apool = p1.enter_context(tc.tile_pool(name="a", bufs=2))
            spool = p1.enter_context(tc.tile_pool(name="sup", bufs=2))
            ppq = p1.enter_context(tc.tile_pool(name="pq", bufs=1, space="PSUM"))
            ppk = p1.enter_context(tc.tile_pool(name="pk", bufs=1, space="PSUM"))
            ppv = p1.enter_context(tc.tile_pool(name="pv", bufs=1, space="PSUM"))
            ppt = p1.enter_context(tc.tile_pool(name="pt", bufs=1, space="PSUM"))
            ppj = p1.enter_context(tc.tile_pool(name="pj", bufs=1, space="PSUM"))
            # tiles whose attention elementwise runs on GpSimd (load-balance
            # vs DVE ~ 3/8); ACT evacuates k/v to SBUF for those.
            GS = set(t for t in range(NT) if t % 8 in (2, 5, 7))
            xt_t2 = None
            aoT_sb = None
            for t in range(NT):
                st = t % 2  # subtile index within supertile
                if st == 0:
                    xt_t2 = dpool.tile([128, 2, 256], F32R, tag="xt")
                    for c in range(2):
                        nc.sync.dma_start(
                            out=xt_t2[:, c, :],
                            in_=xt[c * 128:(c + 1) * 128, t * 128:(t + 2) * 128],
                        )
                # host pre-arranged: free = (n=(m,nn), loc=(r,w))
                kv_t = dpool.tile([128, NW, 128], F32R, tag="kv")
                nc.sync.dma_start(out=kv_t, in_=xc[:, t * 512:(t + 1) * 512])

                # ---- projections on PE (f32r) ----
                q_ps = ppq.tile([128, 264], F32, tag="qps")
                for c in range(2):
                    nc.tensor.matmul(
                        q_ps[:],
                        lhsT=xt_t2[:, c, st * 128:(st + 1) * 128],
                        rhs=wq_sb[:, c, :],
                        start=(c == 0),
                        stop=(c == 1),
                    )
                k_ps = ppk.tile([128, NW, 256], F32, tag="kps")
                v_ps = ppv.tile([128, NW, 256], F32, tag="vps")
                for n in range(NW):
                    nc.tensor.matmul(
                        k_ps[:, n, :], lhsT=kv_t[:, n, :], rhs=wkv_sb[:, 0:256]
                    )
                    nc.tensor.matmul(
                        v_ps[:, n, :], lhsT=kv_t[:, n, :], rhs=wkv_sb[:, 256:512]
                    )

                # ---- q evacuation (+ bias) ----
                q_sb = apool.tile([128, 264], F32, tag="qsb")
                if has_qbias:
                    nc.vector.scalar_tensor_tensor(
                        out=q_sb[:], in0=q_ps[:], scalar=1.0, in1=bq_sb[:],
                        op0=OP.bypass, op1=OP.add,
                    )
                else:
                    nc.scalar.activation(out=q_sb[:], in_=q_ps[:], func=ACT.Copy)

                gs = t in GS
                if gs:
                    # evacuate k/v to SBUF so GpSimd (no PSUM port) can mul
                    k_sb = apool.tile([128, NW, 256], F32, tag="ksb")
                    v_sb = apool.tile([128, NW, 256], F32, tag="vsb")
                    nc.scalar.activation(out=k_sb[:], in_=k_ps[:], func=ACT.Copy)
                    nc.scalar.activation(out=v_sb[:], in_=v_ps[:], func=ACT.Copy)
                    k_in, v_in = k_sb, v_sb
                    eng = nc.gpsimd
                else:
                    k_in, v_in = k_ps, v_ps
                    eng = nc.vector

                # ---- QK: prods = k * q (broadcast over n); fold d then reduce
                prods = apool.tile([128, NW, 256], F32, tag="prods")
                eng.tensor_mul(
                    prods[:], k_in[:], _ap(q_sb[:], [(0, NW), (1, 256)])
                )
                ph = apool.tile([128, NW, G, 16], F32, tag="ph")
                pv4 = prods[:].rearrange("p n (g d) -> p n g d", g=G)
                eng.tensor_add(
                    ph[:],
                    _ap(pv4, [(256, NW), (32, G), (1, 16)]),
                    bass.AP(
                        tensor=pv4.tensor, offset=pv4.offset + 16,
                        ap=[pv4.ap[0], [256, NW], [32, G], [1, 16]],
                    ),
                )
                if gs:
                    ph2 = apool.tile([128, NW, G, 8], F32, tag="ph2")
                    eng.tensor_add(
                        ph2[:],
                        _ap(ph[:], [(128, NW), (16, G), (1, 8)]),
                        bass.AP(
                            tensor=ph[:].tensor, offset=ph[:].offset + 8,
                            ap=[ph[:].ap[0], [128, NW], [16, G], [1, 8]],
                        ),
                    )
                    red_in, red_w = ph2, 8
                else:
                    red_in, red_w = ph, 16
                logits = apool.tile([128, NW, G], F32, tag="logits")
                nc.vector.tensor_reduce(
                    out=logits[:],
                    in_=red_in[:].rearrange("p n g d -> p n g d"),
                    axis=AX.X,
                    op=OP.add,
                )
                if has_cq:
                    nc.vector.scalar_tensor_tensor(
                        out=logits[:], in0=logits[:], scalar=1.0,
                        in1=_ap(q_sb[:, 256:264], [(0, NW), (1, G)]),
                        op0=OP.bypass, op1=OP.add,
                    )

                # ---- softmax_one over n ----
                mx = apool.tile([128, G], F32, tag="mx")
                nc.vector.tensor_reduce(
                    out=mx[:], in_=logits[:].rearrange("p n g -> p g n"),
                    axis=AX.X, op=OP.max,
                )
                em = apool.tile([128, NW, G], F32, tag="em")
                nc.gpsimd.tensor_sub(
                    em[:], logits[:], _ap(mx[:], [(0, NW), (1, G)])
                )
                e = apool.tile([128, NW, G], F32, tag="e")
                nc.scalar.activation(out=e[:], in_=em[:], func=ACT.Exp)
                se = apool.tile([128, G], F32, tag="se")
                nc.vector.tensor_reduce(
                    out=se[:], in_=e[:].rearrange("p n g -> p g n"),
                    axis=AX.X, op=OP.add,
                )
                sp = apool.tile([128, G], F32, tag="sp")
                nc.vector.tensor_scalar_add(sp[:], se[:], 1.0)
                rp = apool.tile([128, G], F32, tag="rp")
                nc.vector.reciprocal(rp[:], sp[:])
                attn = apool.tile([128, NW, G], F32, tag="attn")
                nc.gpsimd.tensor_mul(
                    attn[:], e[:], _ap(rp[:], [(0, NW), (1, G)])
                )

                # ---- AV: prods2 = v * attn (broadcast over d); tree over n --
                prods2 = apool.tile([128, NW, 256], F32, tag="prods2")
                eng.tensor_mul(
                    prods2[:].rearrange("p n (g d) -> p n g d", g=G),
                    v_in[:].rearrange("p n (g d) -> p n g d", g=G),
                    _ap(attn[:], [(G, NW), (1, G), (0, D)]),
                )
                p2h = apool.tile([128, 2, 256], F32, tag="p2h")
                eng.tensor_add(p2h[:], prods2[:, 0:2, :], prods2[:, 2:4, :])
                ao = apool.tile([128, 256], F32, tag="ao")
                eng.tensor_add(ao[:], p2h[:, 0, :], p2h[:, 1, :])
                if has_bv:
                    ssum = apool.tile([128, G], F32, tag="ssum")
                    nc.vector.tensor_mul(ssum[:], se[:], rp[:])
                    bvterm = apool.tile([128, 256], F32, tag="bvterm")
                    nc.vector.tensor_mul(
                        bvterm[:], bv_sb[:], _ap(ssum[:], [(1, G), (0, D)])
                    )
                    nc.vector.tensor_add(ao[:], ao[:], bvterm[:])

                # ---- transpose attention output to feature-major ----
                if st == 0:
                    aoT_sb = spool.tile([128, 2, 256], F32R, tag="aoT")
                aoT_ps = ppt.tile([128, 2, 128], F32, tag="aoTps")
                for c in range(2):
                    nc.tensor.transpose(
                        aoT_ps[:, c, :], ao[:, c * 128:(c + 1) * 128], ident[:]
                    )
                nc.scalar.activation(
                    out=aoT_sb[:, :, st * 128:(st + 1) * 128],
                    in_=aoT_ps[:],
                    func=ACT.Copy,
                )

                # ---- proj + residual (via identity matmul) per supertile ----
                if st == 1:
                    proj_ps = ppj.tile([128, 2, 256], F32, tag="pjps")
                    for co in range(2):
                        for ci in range(2):
                            nc.tensor.matmul(
                                proj_ps[:, co, :],
                                lhsT=wp_sb[:, ci, co, :],
                                rhs=aoT_sb[:, ci, :],
                                start=(ci == 0),
                                stop=False,
                            )
                        nc.tensor.matmul(
                            proj_ps[:, co, :],
                            lhsT=eye_sb[:],
                            rhs=xt_t2[:, co, :],
                            start=False,
                            stop=True,
                        )
                        nc.scalar.activation(
                            out=y_sb[co][:, (t - 1) * 128:(t + 1) * 128],
                            in_=proj_ps[:, co, :],
                            func=ACT.Identity,
                            bias=bp_sb[:, co:co + 1],
                        )
            p1.close()

            # ================= phase 2: adapter MLP ==========================
            with (
                tc.tile_pool(name="p2", bufs=2) as p2pool,
                tc.tile_pool(name="ph", bufs=2, space="PSUM") as pph,
                tc.tile_pool(name="pa2", bufs=2, space="PSUM") as ppa2,
            ):
                for s in range(L // 256):
                    h_ps = pph.tile([DH, 256], F32, tag="hps")
                    for c in range(2):
                        nc.tensor.matmul(
                            h_ps[:],
                            lhsT=wa1_sb[:, c, :],
                            rhs=y_sb[c][:, s * 256:(s + 1) * 256],
                            start=(c == 0),
                            stop=(c == 1),
                        )
                    h_sb = p2pool.tile([DH, 256], F32R, tag="hsb")
                    nc.scalar.activation(
                        out=h_sb[:], in_=h_ps[:], func=ACT.Gelu, bias=ba1_sb[:, 0:1]
                    )
                    a2_ps = ppa2.tile([128, 2, 256], F32, tag="a2ps")
                    for co in range(2):
                        nc.tensor.matmul(
                            a2_ps[:, co, :],
                            lhsT=wa2_sb[:, co, :],
                            rhs=h_sb[:],
                        )
                    o_sb = p2pool.tile([128, 2, 256], F32, tag="osb")
                    for co in range(2):
                        nc.scalar.activation(
                            out=o_sb[:, co, :], in_=a2_ps[:, co, :],
                            func=ACT.Identity, bias=ba2_sb[:, co:co + 1],
                        )
                    for co in range(2):
                        nc.sync.dma_start(
                            out=out[co * 128:(co + 1) * 128, s * 256:(s + 1) * 256],
                            in_=o_sb[:, co, :],
                        )
    nc.finalize()
    return nc


def kernel(x_trans, x_cnn, Wq, bq, Wk, bk, Wv, bv, Wp, bp, Wa1, ba1, Wa2, ba2,
           trace=False):
    scale = float(D) ** -0.5
    wq_s = (Wq * scale).astype(np.float32)
    bq_s = (bq * scale).astype(np.float32)
    # fold of k-bias into logits: cq0 = q . bk per head, computed as 8 extra
    # q-projection columns (x @ Wcq + ccq)
    wcq = np.stack(
        [wq_s[:, g * D:(g + 1) * D] @ bk[g * D:(g + 1) * D] for g in range(G)], axis=1
    ).astype(np.float32)
    ccq = np.array(
        [bq_s[g * D:(g + 1) * D] @ bk[g * D:(g + 1) * D] for g in range(G)],
        dtype=np.float32,
    )
    wq_ext = np.ascontiguousarray(np.concatenate([wq_s, wcq], axis=1))
    bq_ext = np.concatenate([bq_s, ccq])
    wkv = np.ascontiguousarray(np.concatenate([Wk + 0.0, Wv + 0.0], axis=1)).astype(
        np.float32
    )

    has_qbias = bool(np.any(bq_ext != 0.0))
    has_cq = bool(np.any(wcq != 0.0) or np.any(ccq != 0.0))
    has_bv = bool(np.any(bv != 0.0))
    flags = (has_qbias, has_cq, has_bv)
    if flags not in _cache:
        _cache[flags] = _build(flags)
    nc = _cache[flags]

    common = {
        "wq": wq_ext,
        "wkv": wkv,
        "wp": np.ascontiguousarray(Wp, dtype=np.float32),
        "wa1": np.ascontiguousarray(Wa1, dtype=np.float32),
        "wa2": np.ascontiguousarray(Wa2, dtype=np.float32),
        "bqe": np.ascontiguousarray(bq_ext.reshape(264, 1), dtype=np.float32),
        "bvv": np.ascontiguousarray(bv.reshape(CQ, 1), dtype=np.float32),
        "bpv": np.ascontiguousarray(bp.reshape(CQ, 1), dtype=np.float32),
        "ba1v": np.ascontiguousarray(ba1.reshape(DH, 1), dtype=np.float32),
        "ba2v": np.ascontiguousarray(ba2.reshape(CQ, 1), dtype=np.float32),
        "eye": np.eye(128, dtype=np.float32),
    }
    in_maps = []
    for b in range(N_CORES):
        m = dict(common)
        m["xt"] = np.ascontiguousarray(
            x_trans[b].reshape(L, CQ).T, dtype=np.float32
        )
        # [c, hh, ww] -> [c, t, m, nn, r, w]; tile t covers h rows (2t, 2t+1)
        xcb = x_cnn[b].reshape(CKV, 32, 2, 2, 64, 2)  # c, t, r, m, w, nn
        xcb = xcb.transpose(0, 1, 3, 5, 2, 4).reshape(CKV, 4 * L)
        m["xc"] = np.ascontiguousarray(xcb, dtype=np.float32)
        in_maps.append(m)

    res = run_bass_kernel_spmd(
        nc, in_maps, core_ids=list(range(N_CORES)), trace=trace
    )
    kernel.last_results = res
    o = np.stack([r["o"] for r in res.results])  # [8, 256, 4096]
    return (
        o.transpose(0, 2, 1).reshape(8, 64, 64, CQ).astype(np.float32)
    )
